# revision 16
# baseline (speedup 1.0000x reference)
"""Trainium2 Bass kernel for the DLI loss (ragged segment means -> pairwise NLL).

Math reduction used here
------------------------
reference() computes, per batch b:
  states[t] = mean of encoder_output[b, start_t:end_t+1, :]        (ragged turns)
  logits[j,k] = cat(states[j], states[k]) @ W + bias               ([T,T,2])
  loss = mean over pairs k<j of NLL(log_softmax(logits), target=(k==j-1))

With 2 classes only the logit difference matters:
  u[j,k] = A[j] + C[k] + (bias[1]-bias[0])
  A[j] = states[j] . (Wl[:,1]-Wl[:,0]),  C[k] = states[k] . (Wr[:,1]-Wr[:,0])
  nll  = softplus(u) for target 0, softplus(-u) for target 1.

So the only heavy work is the ragged segment SUM of encoder_output (256 MB read),
expressed as a masked matmul: seg[T,D] = M[S,T]^T @ x[S,D] with M a 0/1 segment
membership mask built on-device from iota/compare against the turn end ids.
Each core handles 4 of the 32 batches (pure data parallel), emits the two
dot products (seg . wl, seg . wr) per turn, and the tiny [T,T] softplus
triangle is finished on the host from those 2*T scalars per batch.

bf16 is used for the matmul operands (mask is exact 0/1; x rounds to ~0.2%
per element). The final loss averages 64512 pairs, so the bf16 noise washes
out to ~3e-7 relative error on the scalar output (measured).
"""

import sys
import os

sys.path.insert(0, "/opt/trn_rl_repo")

# The bass kernel executes through PJRT on the axon-tunneled NeuronCores; if a
# caller pinned JAX_PLATFORMS to something without axon (and jax isn't imported
# yet), undo that so jax.devices() can see the 8 cores.
_jp = os.environ.get("JAX_PLATFORMS")
if _jp is not None and "axon" not in _jp and "jax" not in sys.modules:
    del os.environ["JAX_PLATFORMS"]

import numpy as np

# Problem shapes (hardcoded per harness contract).
B, S, D, T = 32, 4096, 512, 64
N_CORES = 8
BPC = B // N_CORES          # batches per core
P = 128                     # SBUF partitions
NH = 2                      # DMA tiles (halves) per batch, 4 MB each
RPP = S // NH // P          # rows per partition per tile (16)
CPW = 4                     # chunks per cast piece (pieces alternate ACT/DVE)
# Position mapping: s = (S//NH)*h + RPP*p + c. Each partition reads one
# contiguous RPP*D*4 = 32 KB block per DMA -> near-peak HBM efficiency.

_PROGRAM_CACHE = {}


def _build_program():
    """Build + compile the per-core Bass/Tile program (identical on all cores)."""
    from contextlib import ExitStack

    import concourse.bacc as bacc
    import concourse.mybir as mybir
    import concourse.tile as tile

    f32 = mybir.dt.float32
    bf16 = mybir.dt.bfloat16

    nc = bacc.Bacc(
        "TRN2", target_bir_lowering=False, debug=False, enable_asserts=False
    )

    x_d = nc.dram_tensor("x", [BPC, S, D], f32, kind="ExternalInput").ap()
    ends_d = nc.dram_tensor("endsb", [BPC, T], f32, kind="ExternalInput").ap()
    wlr_d = nc.dram_tensor("wlr", [2, D], f32, kind="ExternalInput").ap()
    out_d = nc.dram_tensor("out", [T, BPC, 2], f32, kind="ExternalOutput").ap()

    with tile.TileContext(nc) as tc, ExitStack() as ctx:
        singles = ctx.enter_context(tc.tile_pool(name="singles", bufs=1))
        xpool = ctx.enter_context(tc.tile_pool(name="xp", bufs=3))
        bpool = ctx.enter_context(tc.tile_pool(name="bp", bufs=3))
        mpool = ctx.enter_context(tc.tile_pool(name="mp", bufs=2))
        epool = ctx.enter_context(tc.tile_pool(name="ep", bufs=2))
        spool = ctx.enter_context(tc.tile_pool(name="sp", bufs=2))
        ppool = ctx.enter_context(tc.tile_pool(name="pp", bufs=2, space="PSUM"))

        # Per-batch tilings: (start_row, chunks, chunk_offset). Batches 0..2
        # use two 4 MB tiles; the last batch ends with 3 MB + 1 MB tiles so
        # the after-last-DMA chain (cast+matmul) on the critical tail is short.
        STD_TILES = [(0, RPP, 0), (S // NH, RPP, RPP)]
        LAST_TILES = [(0, RPP, 0), (S // NH, 12, RPP), (S // NH + 12 * P, 4, RPP + 12)]
        tilings = [STD_TILES] * (BPC - 1) + [LAST_TILES]
        dma_list = [(b, t) for b in range(BPC) for t in range(len(tilings[b]))]

        def x_dma(b, t):
            row0, ch, _ = tilings[b][t]
            xt = xpool.tile([P, RPP, D], f32, tag="xt")
            nc.sync.dma_start(
                xt[:, :ch, :],
                x_d[b][row0 : row0 + ch * P, :].rearrange("(p c) d -> p c d", c=ch),
            )
            return xt

        # Kick off the first data DMA before any setup work.
        xt_next = x_dma(0, 0)

        NCH = S // P  # 32 chunks per batch
        # Position index tables, replicated along the T axis. A tile starting
        # at row0 with ch chunks holds s = row0 + ch*p + c at [p, c]. iota_t
        # covers the standard (16, 16) tiling, iota3 the last batch's
        # (16, 12, 4) tiling.
        iota_t = singles.tile([P, NCH, T], f32, tag="iota_t")
        iota3 = singles.tile([P, NCH, T], f32, tag="iota3")
        for tgt, tiling in ((iota_t, STD_TILES), (iota3, LAST_TILES)):
            for row0, ch, coff in tiling:
                nc.gpsimd.iota(
                    tgt[:, coff : coff + ch, :],
                    [[1, ch], [0, T]],
                    base=row0,
                    channel_multiplier=ch,
                    allow_small_or_imprecise_dtypes=True,
                )
        # wl/wr difference vectors replicated on T partitions for the row dots.
        wlr_t = singles.tile([T, 2, D], f32)
        nc.sync.dma_start(wlr_t[:], wlr_d.unsqueeze(0).to_broadcast((T, 2, D)))

        out_t = singles.tile([T, BPC, 2], f32)

        dma_iter = iter(dma_list[1:])
        for b in range(BPC):
            # ends on every partition.
            ends_t = epool.tile([P, 1, T], f32)
            nc.sync.dma_start(
                ends_t[:],
                ends_d[b].unsqueeze(0).unsqueeze(0).to_broadcast((P, 1, T)),
            )

            # mask[p,i,t] = (s <= end_t) - (s <= end_{t-1}) in {0,1}, bf16.
            # (s <= end_{t-1}) is just the cmp shifted by one along t.
            iota_src = iota3 if b == BPC - 1 else iota_t
            cmpe = mpool.tile([P, NCH, T], bf16, tag="cmpe")
            mask = mpool.tile([P, NCH, T], bf16, tag="mask")
            nc.vector.tensor_tensor(
                cmpe[:],
                iota_src[:],
                ends_t[:].to_broadcast((P, NCH, T)),
                op=mybir.AluOpType.is_le,
            )
            nc.vector.tensor_sub(
                mask[:, :, 1:], cmpe[:, :, 1:], cmpe[:, :, : T - 1]
            )
            nc.vector.tensor_copy(mask[:, :, 0:1], cmpe[:, :, 0:1])

            psum = ppool.tile([T, D], f32)
            for t, (row0, ch, coff) in enumerate(tilings[b]):
                xt = xt_next
                nxt = next(dma_iter, None)
                if nxt is not None:
                    xt_next = x_dma(*nxt)
                xb = bpool.tile([P, RPP, D], bf16, tag="xb")
                # Cast f32->bf16 in CPW-chunk pieces, alternating ACT/DVE so
                # matmuls start before the whole tile is cast; the final
                # (single-piece) tile casts on the faster DVE.
                npieces = (ch + CPW - 1) // CPW
                last_tile = b == BPC - 1 and t == len(tilings[b]) - 1
                for q in range(npieces):
                    sl = slice(q * CPW, min((q + 1) * CPW, ch))
                    use_act = (q % 2 == 0) and not (last_tile and npieces == 1)
                    eng = nc.scalar.copy if use_act else nc.vector.tensor_copy
                    eng(xb[:, sl, :], xt[:, sl, :])
                    for c in range(sl.start, sl.stop):
                        i = coff + c
                        nc.tensor.matmul(
                            psum[:],
                            mask[:, i, :],
                            xb[:, c, :],
                            start=(i == 0),
                            stop=(i == NCH - 1),
                        )

            # A0[j] = sum_d seg[j,d]*wl[d] ; C0[j] = sum_d seg[j,d]*wr[d]
            # (tensor_tensor_reduce is a custom DVE op that doesn't run on the
            # axon/PJRT path, so use plain mul + reduce.)
            for d_ in range(2):
                scratch = spool.tile([T, D], f32, tag=f"scr{d_}")
                nc.vector.tensor_mul(scratch[:], psum[:], wlr_t[:, d_, :])
                nc.vector.reduce_sum(
                    out_t[:, b, d_ : d_ + 1],
                    scratch[:],
                    axis=mybir.AxisListType.X,
                )

        nc.sync.dma_start(out_d[:], out_t[:])

    nc.compile()
    return nc


def _host_prep(encoder_output, W, b, his_turn_end_ids):
    x = np.ascontiguousarray(np.asarray(encoder_output, dtype=np.float32))
    W = np.asarray(W, dtype=np.float32)
    bias = np.asarray(b, dtype=np.float32)
    ends = np.asarray(his_turn_end_ids).astype(np.int64)

    ends_prev = np.concatenate(
        [np.full((B, 1), -1, np.int64), ends[:, :-1]], axis=1
    )
    endsb = ends.astype(np.float32)  # [B, T]

    wlr = np.stack([W[:D, 1] - W[:D, 0], W[D:, 1] - W[D:, 0]], axis=0)  # [2, D]
    wlr = np.ascontiguousarray(wlr, dtype=np.float32)
    bd = np.float64(np.float32(bias[1]) - np.float32(bias[0]))

    counts = (ends - ends_prev).astype(np.float64)  # [B, T]
    return x, endsb, wlr, bd, counts


def _host_finish(A0, C0, counts, bd):
    """A0/C0: [B, T] raw dots of segment sums; returns the scalar loss."""
    A = A0.astype(np.float64) / counts
    C = C0.astype(np.float64) / counts
    u = A[:, :, None] + C[:, None, :] + bd  # [B, T, T]
    j = np.arange(T)[:, None]
    k = np.arange(T)[None, :]
    tri = k < j
    adj = k == (j - 1)
    nll = np.where(adj, np.logaddexp(0.0, -u), np.logaddexp(0.0, u))
    n_pairs = B * (T * (T - 1) // 2)
    loss = np.sum(np.where(tri, nll, 0.0)) / n_pairs
    return np.asarray(loss, dtype=np.float32)


def kernel(encoder_output, W, b, his_turn_end_ids):
    from concourse.bass_utils import run_bass_kernel_spmd

    x, endsb, wlr, bd, counts = _host_prep(encoder_output, W, b, his_turn_end_ids)

    if "nc" not in _PROGRAM_CACHE:
        _PROGRAM_CACHE["nc"] = _build_program()
    nc = _PROGRAM_CACHE["nc"]

    in_maps = [
        {
            "x": x[i * BPC : (i + 1) * BPC],
            "endsb": endsb[i * BPC : (i + 1) * BPC],
            "wlr": wlr,
        }
        for i in range(N_CORES)
    ]
    trace = bool(int(os.environ.get("BASS_KERNEL_TRACE", "0")))
    kw = {}
    if os.environ.get("BASS_KERNEL_TMPDIR"):
        kw["tmpdir"] = os.environ["BASS_KERNEL_TMPDIR"]
    res = run_bass_kernel_spmd(nc, in_maps, list(range(N_CORES)), trace=trace, **kw)
    _PROGRAM_CACHE["last_results"] = res

    # per-core out: [T, BPC, 2] -> A0/C0 [B, T]
    A0 = np.concatenate([r["out"][:, :, 0].T for r in res.results], axis=0)
    C0 = np.concatenate([r["out"][:, :, 1].T for r in res.results], axis=0)
    return _host_finish(A0, C0, counts, bd)
